# revision 15
# baseline (speedup 1.0000x reference)
"""Trainium2 Bass kernel: GroupNorm + 8-head self-attention + proj + residual.

Problem: x[8, 512, 32, 32]; per batch b:
  xn = GroupNorm(x_b) (8 groups over C=512, eps=1e-5) * gn_w + gn_b
  q/k/v = xn @ w{q,k,v}.T + b{q,k,v}   (heads=8, d=64)
  att   = softmax(q k^T / 8) v ; out = att @ wp.T + bp + x_b

Sharding: pure data-parallel over batch (8 batches -> 8 cores), no collectives.

Per-core layouts (partition x free):
  x, xn slabs    [128, 4, 1024]   channel-major (c = 128*j + p), s free
  qT, kT slabs   [128, 4, 1024]   row c_out = head*64+e, col = token
  v slab         [128, 8, 520]    [m-tile][token-in-tile, head*65 + e], col 64 of
                                  each 65-block is ones (gives Z row in att@v)
  scores^T       PSUM [128, 1024] (k-token m on partitions, q-token n free)
  exp via ACT (no max subtraction; scores ~ N(0,1) after the 1/8 scale)
  att@v: out[65, n] = [v_h | 1]^T @ PT  -> rows 0..63 = U^T, row 64 = Z
  normalize: U / Z via reciprocal + DMA partition-broadcast + DVE mul
  proj back into [128, 4, 1024] channel-major, + bias + residual, DMA out.
"""

import os
import sys

sys.path.insert(0, "/opt/trn_rl_repo")
os.environ.setdefault("MYCRO_LOCAL_CACHE", "1")

import numpy as np

import concourse.bass as bass
import concourse.tile as tile
from concourse import bacc, mybir
from concourse.bass_utils import run_bass_kernel_spmd

C = 512
S = 1024
NT = 4        # channel tiles of 128
MT = 8        # token tiles of 128
NH = 8        # heads
D = 64        # head dim
GROUPS = 8
EPS = 1e-5
N_CORES = 8
GRP_N = (C // GROUPS) * S  # elements per group = 64 * 1024

F32 = mybir.dt.float32
F32R = mybir.dt.float32r
BF16 = mybir.dt.bfloat16

# matmul input dtype mode: "f32r" (full-precision fp32 @ 1 cyc/row), "f32"
# (4 cyc/row), or "bf16"
MM_MODE = os.environ.get("KERNEL_MM_MODE", "f32r")


MMDT = {"f32r": F32R, "f32": F32, "bf16": BF16}[MM_MODE]


def build_nc():
    nc = bacc.Bacc("TRN2", target_bir_lowering=False, debug=False,
                   num_devices=N_CORES)

    x_d = nc.dram_tensor("x", [C, S], F32, kind="ExternalInput")
    wqT_d = nc.dram_tensor("wqT", [C, C], MMDT, kind="ExternalInput")
    wkT_d = nc.dram_tensor("wkT", [C, C], MMDT, kind="ExternalInput")
    wvT_d = nc.dram_tensor("wvT", [C, C], MMDT, kind="ExternalInput")
    wpT_d = nc.dram_tensor("wpT", [C, C], MMDT, kind="ExternalInput")
    gnw_d = nc.dram_tensor("gnw4", [128, NT], F32, kind="ExternalInput")
    gnb_d = nc.dram_tensor("gnb4", [128, NT], F32, kind="ExternalInput")
    bq_d = nc.dram_tensor("bq4", [128, NT], F32, kind="ExternalInput")
    bk_d = nc.dram_tensor("bk4", [128, NT], F32, kind="ExternalInput")
    bp_d = nc.dram_tensor("bp4", [128, NT], F32, kind="ExternalInput")
    bv_d = nc.dram_tensor("bv", [1, C], MMDT, kind="ExternalInput")
    ones1_d = nc.dram_tensor("ones1", [1, 128], MMDT, kind="ExternalInput")
    vones_d = nc.dram_tensor("vones", [128, MT * NH], MMDT, kind="ExternalInput")
    out_d = nc.dram_tensor("out", [C, S], F32, kind="ExternalOutput")

    # group indicator: gind[p, a] = 1 if p // 64 == a  (group of channel
    # c = 128j + p is g = 2j + p // 64)
    gind_np = np.zeros((128, 2), np.float32)
    gind_np[0:64, 0] = 1.0
    gind_np[64:128, 1] = 1.0
    gind_d = nc.inline_tensor(gind_np, name="gind")
    g2_d = nc.inline_tensor(np.ascontiguousarray(gind_np.T), name="g2")

    with tile.TileContext(nc) as tc:
        _emit(tc, x_d, wqT_d, wkT_d, wvT_d, wpT_d, gnw_d, gnb_d,
              bq_d, bk_d, bp_d, bv_d, gind_d, g2_d, out_d, ones1_d, vones_d)
    nc.compile()
    return nc


def _emit(tc, x_d, wqT_d, wkT_d, wvT_d, wpT_d, gnw_d, gnb_d,
          bq_d, bk_d, bp_d, bv_d, gind_d, g2_d, out_d, ones1_d, vones_d):
    from contextlib import ExitStack

    nc = tc.nc
    with ExitStack() as ctx:
        const = ctx.enter_context(tc.tile_pool(name="const", bufs=1))
        slabs = ctx.enter_context(tc.tile_pool(name="slabs", bufs=1))
        scratch = ctx.enter_context(tc.tile_pool(name="scratch", bufs=2))
        pt_pool = ctx.enter_context(tc.tile_pool(name="pt", bufs=3))
        zb_pool = ctx.enter_context(tc.tile_pool(name="zb", bufs=2))
        ps_mm = ctx.enter_context(tc.tile_pool(name="ps_mm", bufs=2, space="PSUM"))
        ps_av = ctx.enter_context(tc.tile_pool(name="ps_av", bufs=2, space="PSUM"))

        # ---- constant / parameter loads ----
        gnw4 = const.tile([128, NT], F32)
        gnb4 = const.tile([128, NT], F32)
        bq4 = const.tile([128, NT], F32)
        bk4 = const.tile([128, NT], F32)
        bp4 = const.tile([128, NT], F32)
        bvr = const.tile([1, C], MMDT)
        gind = const.tile([128, 2], F32)
        g2s = const.tile([2, 128], F32)
        ones_r = const.tile([1, 128], MMDT)
        eps2 = const.tile([2, 1], F32)
        nc.sync.dma_start(out=gnw4, in_=gnw_d[:, :])
        nc.sync.dma_start(out=gnb4, in_=gnb_d[:, :])
        nc.sync.dma_start(out=bq4, in_=bq_d[:, :])
        nc.sync.dma_start(out=bk4, in_=bk_d[:, :])
        nc.sync.dma_start(out=bp4, in_=bp_d[:, :])
        nc.sync.dma_start(out=bvr, in_=bv_d[:, :])
        nc.sync.dma_start(out=gind, in_=gind_d[:, :])
        nc.sync.dma_start(out=g2s, in_=g2_d[:, :])
        nc.sync.dma_start(out=ones_r, in_=ones1_d[:, :])
        nc.vector.memset(eps2, EPS)

        # ---- big slabs ----
        xs = slabs.tile([128, NT, S], F32)     # input, kept for residual
        xn = slabs.tile([128, NT, S], MMDT)     # groupnormed
        qT = slabs.tile([128, NT, S], MMDT)
        kT = slabs.tile([128, NT, S], MMDT)
        vs = slabs.tile([128, MT, NH * 65], MMDT)
        att = slabs.tile([128, NT, S], MMDT)   # attention output (c-major)
        outs = slabs.tile([128, NT, S], F32)
        wq = slabs.tile([128, NT, C], MMDT)     # wqT[c_in, c_out] c_in-major
        wk = slabs.tile([128, NT, C], MMDT)
        wv = slabs.tile([128, NT, C], MMDT)
        wp = slabs.tile([128, NT, C], MMDT)

        for j in range(NT):
            nc.sync.dma_start(out=xs[:, j, :], in_=x_d[128 * j:128 * (j + 1), :])
        for w_sb, w_dr in ((wq, wqT_d), (wk, wkT_d), (wv, wvT_d), (wp, wpT_d)):
            for j in range(NT):
                nc.sync.dma_start(out=w_sb[:, j, :],
                                  in_=w_dr[128 * j:128 * (j + 1), :])

        # ones column in each 65-wide head block of the v slab
        vs5 = vs[:, :, :].rearrange("p m (h e) -> p m h e", e=65)
        nc.sync.dma_start(out=vs5[:, :, :, 64:65],
                          in_=vones_d[:, :].rearrange("p (m h) -> p m h", h=NH))

        # ---- GroupNorm statistics ----
        stat = const.tile([128, 8], F32)   # cols 0..3 sums, 4..7 sumsqs
        for j in range(NT):
            nc.vector.reduce_sum(out=stat[:, j:j + 1], in_=xs[:, j, :],
                                 axis=mybir.AxisListType.X)
            sq = scratch.tile([128, S], F32, tag="sq")
            nc.scalar.activation(out=sq, in_=xs[:, j, :],
                                 func=mybir.ActivationFunctionType.Square,
                                 accum_out=stat[:, 4 + j:5 + j])

        gps = ps_av.tile([2, 8], F32, tag="av")
        nc.tensor.matmul(gps[:, :], gind[:, :], stat[:, :], start=True, stop=True)
        gs = const.tile([2, 8], F32)
        nc.vector.tensor_copy(out=gs, in_=gps[:, :])

        mv2 = const.tile([2, 8], F32)          # cols 0..3 mean, 4..7 rstd
        var4 = const.tile([2, 4], F32)
        msq4 = const.tile([2, 4], F32)
        nc.scalar.mul(out=mv2[:, 0:4], in_=gs[:, 0:4], mul=1.0 / GRP_N)
        nc.scalar.mul(out=var4, in_=gs[:, 4:8], mul=1.0 / GRP_N)
        nc.vector.tensor_mul(out=msq4, in0=mv2[:, 0:4], in1=mv2[:, 0:4])
        nc.vector.tensor_tensor(out=var4, in0=var4, in1=msq4,
                                op=mybir.AluOpType.subtract)
        nc.scalar.activation(out=var4, in_=var4,
                             func=mybir.ActivationFunctionType.Sqrt,
                             bias=eps2[:, :])
        nc.vector.reciprocal(out=mv2[:, 4:8], in_=var4)

        bcp = ps_av.tile([128, 8], F32, tag="av")
        nc.tensor.matmul(bcp[:, :], g2s[:, :], mv2[:, :], start=True, stop=True)
        a4 = const.tile([128, NT], F32)
        b4 = const.tile([128, NT], F32)
        tmp4 = const.tile([128, NT], F32)
        nc.vector.tensor_mul(out=a4, in0=bcp[:, 4:8], in1=gnw4)
        nc.vector.tensor_mul(out=tmp4, in0=bcp[:, 0:4], in1=a4)
        nc.vector.tensor_tensor(out=b4, in0=gnb4, in1=tmp4,
                                op=mybir.AluOpType.subtract)
        for j in range(NT):
            nc.vector.tensor_scalar(out=xn[:, j, :], in0=xs[:, j, :],
                                    scalar1=a4[:, j:j + 1], scalar2=b4[:, j:j + 1],
                                    op0=mybir.AluOpType.mult,
                                    op1=mybir.AluOpType.add)

        # ---- Q/K projections:  qT[c_out, n] = sum_c wqT[c, c_out] xn[c, n] ----
        for w_sb, bias4, dst in ((wq, bq4, qT), (wk, bk4, kT)):
            for tt in range(NT):
                ps = ps_mm.tile([128, S], F32, tag="mm")
                for ch in range(2):
                    for kk in range(NT):
                        nc.tensor.matmul(
                            ps[:, 512 * ch:512 * (ch + 1)],
                            w_sb[:, kk, 128 * tt:128 * (tt + 1)],
                            xn[:, kk, 512 * ch:512 * (ch + 1)],
                            start=(kk == 0), stop=(kk == NT - 1))
                nc.vector.tensor_scalar_add(out=dst[:, tt, :], in0=ps[:, :],
                                            scalar1=bias4[:, tt:tt + 1])

        # ---- V projection (token-major):  v[m, c_out] = sum_c xn[c, m] wvT[c, c_out]
        vs4 = vs[:, :, :].rearrange("p m (h e) -> p m h e", e=65)
        for mt in range(MT):
            ps = ps_mm.tile([128, 512], F32, tag="mm")
            for kk in range(NT):
                nc.tensor.matmul(ps[:, :],
                                 xn[:, kk, 128 * mt:128 * (mt + 1)],
                                 wv[:, kk, :],
                                 start=(kk == 0), stop=False)
            nc.tensor.matmul(ps[:, :], ones_r[:, :], bvr[:, :],
                             start=False, stop=True)
            nc.vector.tensor_copy(
                out=vs4[:, mt, :, 0:64],
                in_=ps.rearrange("p (h e) -> p h e", e=64))

        # ---- attention per head ----
        for h in range(NH):
            tt = h // 2
            p0 = 64 * (h % 2)
            qh = qT[p0:p0 + 64, tt, :]
            pav = ps_av.tile([65, S], F32, tag="av")
            for mt in range(MT):
                pss = ps_mm.tile([128, S], F32, tag="mm")
                for ch in range(2):
                    nc.tensor.matmul(
                        pss[:, 512 * ch:512 * (ch + 1)],
                        kT[p0:p0 + 64, tt, 128 * mt:128 * (mt + 1)],
                        qh[:, 512 * ch:512 * (ch + 1)],
                        start=True, stop=True)
                pt = pt_pool.tile([128, S], MMDT, tag="pt")
                nc.scalar.activation(out=pt, in_=pss[:, :],
                                     func=mybir.ActivationFunctionType.Exp,
                                     scale=0.125)
                for ch in range(2):
                    nc.tensor.matmul(
                        pav[:, 512 * ch:512 * (ch + 1)],
                        vs[:, mt, 65 * h:65 * h + 65],
                        pt[:, 512 * ch:512 * (ch + 1)],
                        start=(mt == 0), stop=(mt == MT - 1),
                        skip_group_check=True)
            # normalize: att[head rows, n] = U[e, n] * (1 / Z[n])
            rz = scratch.tile([1, S], MMDT, tag="rz")
            with nc.allow_low_precision(reason="1/Z rounded to matmul dtype"):
                nc.vector.reciprocal(out=rz, in_=pav[64:65, :])
            zb = ps_mm.tile([64, S], F32, tag="mm")
            for ch in range(2):
                nc.tensor.matmul(zb[:, 512 * ch:512 * (ch + 1)],
                                 ones_r[0:1, 0:64],
                                 rz[0:1, 512 * ch:512 * (ch + 1)],
                                 start=True, stop=True)
            zbs = zb_pool.tile([64, S], F32, tag="zb")
            nc.vector.tensor_copy(out=zbs, in_=zb[:, :])
            nc.vector.tensor_mul(out=att[p0:p0 + 64, tt, :],
                                 in0=pav[0:64, :], in1=zbs)

        # ---- output projection + bias + residual ----
        for tt in range(NT):
            ps = ps_mm.tile([128, S], F32, tag="mm")
            for ch in range(2):
                for kk in range(NT):
                    nc.tensor.matmul(
                        ps[:, 512 * ch:512 * (ch + 1)],
                        wp[:, kk, 128 * tt:128 * (tt + 1)],
                        att[:, kk, 512 * ch:512 * (ch + 1)],
                        start=(kk == 0), stop=(kk == NT - 1))
            nc.scalar.activation(out=outs[:, tt, :], in_=ps[:, :],
                                 func=mybir.ActivationFunctionType.Identity,
                                 bias=bp4[:, tt:tt + 1])
            nc.vector.tensor_tensor(out=outs[:, tt, :], in0=outs[:, tt, :],
                                    in1=xs[:, tt, :], op=mybir.AluOpType.add)
            nc.sync.dma_start(out=out_d[128 * tt:128 * (tt + 1), :],
                              in_=outs[:, tt, :])


_NC_CACHE = {}


def _get_nc():
    key = MM_MODE
    if key not in _NC_CACHE:
        _NC_CACHE[key] = build_nc()
    return _NC_CACHE[key]


def _host_prep(x, gn_w, gn_b, wq, bq, wk, bk, wv, bv, wp, bp):
    x = np.asarray(x, np.float32)
    B = x.shape[0]
    x_flat = np.ascontiguousarray(x.reshape(B, C, S))

    def col4(v):
        return np.ascontiguousarray(np.asarray(v, np.float32).reshape(NT, 128).T)

    shared = {
        "wqT": np.ascontiguousarray(np.asarray(wq, np.float32).T),
        "wkT": np.ascontiguousarray(np.asarray(wk, np.float32).T),
        "wvT": np.ascontiguousarray(np.asarray(wv, np.float32).T),
        "wpT": np.ascontiguousarray(np.asarray(wp, np.float32).T),
        "gnw4": col4(gn_w),
        "gnb4": col4(gn_b),
        "bq4": col4(bq),
        "bk4": col4(bk),
        "bp4": col4(bp),
        "bv": np.ascontiguousarray(np.asarray(bv, np.float32).reshape(1, C)),
        "ones1": np.ones((1, 128), np.float32),
        "vones": np.ones((128, MT * NH), np.float32),
    }
    in_maps = [dict(shared, x=x_flat[i]) for i in range(B)]
    return in_maps


def kernel(x, gn_w, gn_b, wq, bq, wk, bk, wv, bv, wp, bp, **kw):
    nc = _get_nc()
    in_maps = _host_prep(x, gn_w, gn_b, wq, bq, wk, bk, wv, bv, wp, bp)
    res = run_bass_kernel_spmd(nc, in_maps, list(range(N_CORES)), **kw)
    out = np.stack([res.results[i]["out"] for i in range(N_CORES)])
    return out.reshape(8, C, 32, 32).astype(np.float32)


if __name__ == "__main__":
    rng = np.random.default_rng(0)
    xs_ = rng.standard_normal((8, C, 32, 32), dtype=np.float32)
    print("built nc ok:", _get_nc())


# revision 17
# speedup vs baseline: 1.4971x; 1.4971x over previous
"""Trainium2 Bass kernel: GroupNorm + 8-head self-attention + proj + residual.

Problem: x[8, 512, 32, 32]; per batch b:
  xn = GroupNorm(x_b) (8 groups over C=512, eps=1e-5) * gn_w + gn_b
  q/k/v = xn @ w{q,k,v}.T + b{q,k,v}   (heads=8, d=64)
  att   = softmax(q k^T / 8) v ; out = att @ wp.T + bp + x_b

Sharding: pure data-parallel over batch (8 batches -> 8 cores), no collectives.

Per-core layouts (partition x free):
  x, xn slabs    [128, 4, 1024]   channel-major (c = 128*j + p), s free
  qT, kT slabs   [128, 4, 1024]   row c_out = head*64+e, col = token
  v slab         [128, 8, 520]    [m-tile][token-in-tile, head*65 + e], col 64 of
                                  each 65-block is ones (gives Z row in att@v)
  scores^T       PSUM [128, 1024] (k-token m on partitions, q-token n free)
  exp via ACT (no max subtraction; scores ~ N(0,1) after the 1/8 scale)
  att@v: out[65, n] = [v_h | 1]^T @ PT  -> rows 0..63 = U^T, row 64 = Z
  normalize: U / Z via reciprocal + DMA partition-broadcast + DVE mul
  proj back into [128, 4, 1024] channel-major, + bias + residual, DMA out.
"""

import os
import sys

sys.path.insert(0, "/opt/trn_rl_repo")
os.environ.setdefault("MYCRO_LOCAL_CACHE", "1")

import numpy as np

import concourse.bass as bass
import concourse.tile as tile
from concourse import bacc, mybir
from concourse.bass_utils import run_bass_kernel_spmd

C = 512
S = 1024
NT = 4        # channel tiles of 128
MT = 8        # token tiles of 128
NH = 8        # heads
D = 64        # head dim
GROUPS = 8
EPS = 1e-5
N_CORES = 8
GRP_N = (C // GROUPS) * S  # elements per group = 64 * 1024

F32 = mybir.dt.float32
F32R = mybir.dt.float32r
BF16 = mybir.dt.bfloat16

# matmul input dtype mode: "f32r" (full-precision fp32 @ 1 cyc/row), "f32"
# (4 cyc/row), or "bf16"
MM_MODE = os.environ.get("KERNEL_MM_MODE", "f32r")


MMDT = {"f32r": F32R, "f32": F32, "bf16": BF16}[MM_MODE]


def build_nc():
    nc = bacc.Bacc("TRN2", target_bir_lowering=False, debug=False,
                   num_devices=N_CORES)

    x_d = nc.dram_tensor("x", [C, S], F32, kind="ExternalInput")
    wqT_d = nc.dram_tensor("wqT", [C, C], MMDT, kind="ExternalInput")
    wkT_d = nc.dram_tensor("wkT", [C, C], MMDT, kind="ExternalInput")
    wvT_d = nc.dram_tensor("wvT", [C, C], MMDT, kind="ExternalInput")
    wpT_d = nc.dram_tensor("wpT", [C, C], MMDT, kind="ExternalInput")
    gnw_d = nc.dram_tensor("gnw4", [128, NT], F32, kind="ExternalInput")
    gnb_d = nc.dram_tensor("gnb4", [128, NT], F32, kind="ExternalInput")
    bq_d = nc.dram_tensor("bq4", [128, NT], F32, kind="ExternalInput")
    bk_d = nc.dram_tensor("bk4", [128, NT], F32, kind="ExternalInput")
    bp_d = nc.dram_tensor("bp4", [128, NT], F32, kind="ExternalInput")
    bv_d = nc.dram_tensor("bv", [1, C], F32, kind="ExternalInput")
    vones_d = nc.dram_tensor("vones", [128, MT * NH], MMDT, kind="ExternalInput")
    out_d = nc.dram_tensor("out", [C, S], F32, kind="ExternalOutput")

    # group indicator: gind[p, a] = 1 if p // 64 == a  (group of channel
    # c = 128j + p is g = 2j + p // 64)
    gind_np = np.zeros((128, 2), np.float32)
    gind_np[0:64, 0] = 1.0
    gind_np[64:128, 1] = 1.0
    gind_d = nc.inline_tensor(gind_np, name="gind")
    g2_d = nc.inline_tensor(np.ascontiguousarray(gind_np.T), name="g2")

    with tile.TileContext(nc) as tc:
        _emit(tc, x_d, wqT_d, wkT_d, wvT_d, wpT_d, gnw_d, gnb_d,
              bq_d, bk_d, bp_d, bv_d, gind_d, g2_d, out_d, vones_d)
    nc.compile()
    return nc


def _emit(tc, x_d, wqT_d, wkT_d, wvT_d, wpT_d, gnw_d, gnb_d,
          bq_d, bk_d, bp_d, bv_d, gind_d, g2_d, out_d, vones_d):
    from contextlib import ExitStack

    nc = tc.nc
    with ExitStack() as ctx:
        const = ctx.enter_context(tc.tile_pool(name="const", bufs=1))
        slabs = ctx.enter_context(tc.tile_pool(name="slabs", bufs=1))
        scratch = ctx.enter_context(tc.tile_pool(name="scratch", bufs=2))
        pt_pool = ctx.enter_context(tc.tile_pool(name="pt", bufs=3))
        zb_pool = ctx.enter_context(tc.tile_pool(name="zb", bufs=2))
        ps_mm = ctx.enter_context(tc.tile_pool(name="ps_mm", bufs=2, space="PSUM"))
        ps_av = ctx.enter_context(tc.tile_pool(name="ps_av", bufs=2, space="PSUM"))

        # ---- constant / parameter loads ----
        gnw4 = const.tile([128, NT], F32)
        gnb4 = const.tile([128, NT], F32)
        bq4 = const.tile([128, NT], F32)
        bk4 = const.tile([128, NT], F32)
        bp4 = const.tile([128, NT], F32)
        bvr = const.tile([1, C], F32)
        bvb = const.tile([128, C], F32)
        gind = const.tile([128, 2], F32)
        g2s = const.tile([2, 128], F32)
        eps2 = const.tile([2, 1], F32)
        nc.sync.dma_start(out=gnw4, in_=gnw_d[:, :])
        nc.sync.dma_start(out=gnb4, in_=gnb_d[:, :])
        nc.sync.dma_start(out=bq4, in_=bq_d[:, :])
        nc.sync.dma_start(out=bk4, in_=bk_d[:, :])
        nc.sync.dma_start(out=bp4, in_=bp_d[:, :])
        nc.sync.dma_start(out=bvr, in_=bv_d[:, :])
        nc.gpsimd.partition_broadcast(out_ap=bvb[:, :], in_ap=bvr[:, :])
        nc.sync.dma_start(out=gind, in_=gind_d[:, :])
        nc.sync.dma_start(out=g2s, in_=g2_d[:, :])
        nc.vector.memset(eps2, EPS)

        # ---- big slabs ----
        xs = slabs.tile([128, NT, S], F32)     # input, kept for residual
        xn = slabs.tile([128, NT, S], MMDT)     # groupnormed
        qT = slabs.tile([128, NT, S], MMDT)
        kT = slabs.tile([128, NT, S], MMDT)
        vs = slabs.tile([128, MT, NH * 65], MMDT)
        att = slabs.tile([128, NT, S], MMDT)   # attention output (c-major)
        outs = slabs.tile([128, NT, S], F32)
        wq = slabs.tile([128, NT, C], MMDT)     # wqT[c_in, c_out] c_in-major
        wk = slabs.tile([128, NT, C], MMDT)
        wv = slabs.tile([128, NT, C], MMDT)
        wp = slabs.tile([128, NT, C], MMDT)

        for j in range(NT):
            nc.sync.dma_start(out=xs[:, j, :], in_=x_d[128 * j:128 * (j + 1), :])
        for w_sb, w_dr in ((wq, wqT_d), (wk, wkT_d), (wv, wvT_d), (wp, wpT_d)):
            for j in range(NT):
                nc.sync.dma_start(out=w_sb[:, j, :],
                                  in_=w_dr[128 * j:128 * (j + 1), :])

        # ones column in each 65-wide head block of the v slab
        vs5 = vs[:, :, :].rearrange("p m (h e) -> p m h e", e=65)
        nc.sync.dma_start(out=vs5[:, :, :, 64:65],
                          in_=vones_d[:, :].rearrange("p (m h) -> p m h", h=NH))

        # ---- GroupNorm statistics ----
        stat = const.tile([128, 8], F32)   # cols 0..3 sums, 4..7 sumsqs
        for j in range(NT):
            nc.vector.reduce_sum(out=stat[:, j:j + 1], in_=xs[:, j, :],
                                 axis=mybir.AxisListType.X)
            sq = scratch.tile([128, S], F32, tag="sq")
            nc.scalar.activation(out=sq, in_=xs[:, j, :],
                                 func=mybir.ActivationFunctionType.Square,
                                 accum_out=stat[:, 4 + j:5 + j])

        gps = ps_av.tile([2, 8], F32, tag="av")
        nc.tensor.matmul(gps[:, :], gind[:, :], stat[:, :], start=True, stop=True)
        gs = const.tile([2, 8], F32)
        nc.vector.tensor_copy(out=gs, in_=gps[:, :])

        mv2 = const.tile([2, 8], F32)          # cols 0..3 mean, 4..7 rstd
        var4 = const.tile([2, 4], F32)
        msq4 = const.tile([2, 4], F32)
        nc.scalar.mul(out=mv2[:, 0:4], in_=gs[:, 0:4], mul=1.0 / GRP_N)
        nc.scalar.mul(out=var4, in_=gs[:, 4:8], mul=1.0 / GRP_N)
        nc.vector.tensor_mul(out=msq4, in0=mv2[:, 0:4], in1=mv2[:, 0:4])
        nc.vector.tensor_tensor(out=var4, in0=var4, in1=msq4,
                                op=mybir.AluOpType.subtract)
        nc.scalar.activation(out=var4, in_=var4,
                             func=mybir.ActivationFunctionType.Sqrt,
                             bias=eps2[:, :])
        nc.vector.reciprocal(out=mv2[:, 4:8], in_=var4)

        bcp = ps_av.tile([128, 8], F32, tag="av")
        nc.tensor.matmul(bcp[:, :], g2s[:, :], mv2[:, :], start=True, stop=True)
        a4 = const.tile([128, NT], F32)
        b4 = const.tile([128, NT], F32)
        tmp4 = const.tile([128, NT], F32)
        nc.vector.tensor_mul(out=a4, in0=bcp[:, 4:8], in1=gnw4)
        nc.vector.tensor_mul(out=tmp4, in0=bcp[:, 0:4], in1=a4)
        nc.vector.tensor_tensor(out=b4, in0=gnb4, in1=tmp4,
                                op=mybir.AluOpType.subtract)
        for j in range(NT):
            nc.vector.tensor_scalar(out=xn[:, j, :], in0=xs[:, j, :],
                                    scalar1=a4[:, j:j + 1], scalar2=b4[:, j:j + 1],
                                    op0=mybir.AluOpType.mult,
                                    op1=mybir.AluOpType.add)

        # ---- Q/K projections:  qT[c_out, n] = sum_c wqT[c, c_out] xn[c, n] ----
        for w_sb, bias4, dst in ((wq, bq4, qT), (wk, bk4, kT)):
            for tt in range(NT):
                ps = ps_mm.tile([128, S], F32, tag="mm")
                for ch in range(2):
                    for kk in range(NT):
                        nc.tensor.matmul(
                            ps[:, 512 * ch:512 * (ch + 1)],
                            w_sb[:, kk, 128 * tt:128 * (tt + 1)],
                            xn[:, kk, 512 * ch:512 * (ch + 1)],
                            start=(kk == 0), stop=(kk == NT - 1))
                nc.vector.tensor_scalar_add(out=dst[:, tt, :], in0=ps[:, :],
                                            scalar1=bias4[:, tt:tt + 1])

        # ---- V projection (token-major):  v[m, c_out] = sum_c xn[c, m] wvT[c, c_out]
        vs4 = vs[:, :, :].rearrange("p m (h e) -> p m h e", e=65)
        for mt in range(MT):
            ps = ps_mm.tile([128, 512], F32, tag="mm")
            for kk in range(NT):
                nc.tensor.matmul(ps[:, :],
                                 xn[:, kk, 128 * mt:128 * (mt + 1)],
                                 wv[:, kk, :],
                                 start=(kk == 0), stop=(kk == NT - 1))
            nc.vector.tensor_tensor(
                out=vs4[:, mt, :, 0:64],
                in0=ps.rearrange("p (h e) -> p h e", e=64),
                in1=bvb.rearrange("p (h e) -> p h e", e=64),
                op=mybir.AluOpType.add)

        # ---- attention per head ----
        for h in range(NH):
            tt = h // 2
            p0 = 64 * (h % 2)
            qh = qT[p0:p0 + 64, tt, :]
            pav = ps_av.tile([65, S], F32, tag="av")
            for mt in range(MT):
                pss = ps_mm.tile([128, S], F32, tag="mm")
                for ch in range(2):
                    nc.tensor.matmul(
                        pss[:, 512 * ch:512 * (ch + 1)],
                        kT[p0:p0 + 64, tt, 128 * mt:128 * (mt + 1)],
                        qh[:, 512 * ch:512 * (ch + 1)],
                        start=True, stop=True)
                pt = pt_pool.tile([128, S], MMDT, tag="pt")
                nc.scalar.activation(out=pt, in_=pss[:, :],
                                     func=mybir.ActivationFunctionType.Exp,
                                     scale=0.125)
                for ch in range(2):
                    nc.tensor.matmul(
                        pav[:, 512 * ch:512 * (ch + 1)],
                        vs[:, mt, 65 * h:65 * h + 65],
                        pt[:, 512 * ch:512 * (ch + 1)],
                        start=(mt == 0), stop=(mt == MT - 1),
                        skip_group_check=True)
            # normalize: att[head rows, n] = U[e, n] * (1 / Z[n])
            zrow = scratch.tile([1, S], F32, tag="zrow")
            nc.vector.tensor_copy(out=zrow, in_=pav[64:65, :])
            rz = scratch.tile([1, S], F32, tag="rz")
            nc.vector.reciprocal_approx_fast(out=rz[:, :], in_=zrow[:, :])
            zbs = zb_pool.tile([64, S], F32, tag="zb")
            nc.gpsimd.partition_broadcast(out_ap=zbs[:, :], in_ap=rz[:, :])
            nc.vector.tensor_mul(out=att[p0:p0 + 64, tt, :],
                                 in0=pav[0:64, :], in1=zbs)

        # ---- output projection + bias + residual ----
        for tt in range(NT):
            ps = ps_mm.tile([128, S], F32, tag="mm")
            for ch in range(2):
                for kk in range(NT):
                    nc.tensor.matmul(
                        ps[:, 512 * ch:512 * (ch + 1)],
                        wp[:, kk, 128 * tt:128 * (tt + 1)],
                        att[:, kk, 512 * ch:512 * (ch + 1)],
                        start=(kk == 0), stop=(kk == NT - 1))
            nc.scalar.activation(out=outs[:, tt, :], in_=ps[:, :],
                                 func=mybir.ActivationFunctionType.Identity,
                                 bias=bp4[:, tt:tt + 1])
            nc.vector.tensor_tensor(out=outs[:, tt, :], in0=outs[:, tt, :],
                                    in1=xs[:, tt, :], op=mybir.AluOpType.add)
            nc.sync.dma_start(out=out_d[128 * tt:128 * (tt + 1), :],
                              in_=outs[:, tt, :])


_NC_CACHE = {}


def _get_nc():
    key = MM_MODE
    if key not in _NC_CACHE:
        _NC_CACHE[key] = build_nc()
    return _NC_CACHE[key]


def _host_prep(x, gn_w, gn_b, wq, bq, wk, bk, wv, bv, wp, bp):
    x = np.asarray(x, np.float32)
    B = x.shape[0]
    x_flat = np.ascontiguousarray(x.reshape(B, C, S))

    def col4(v):
        return np.ascontiguousarray(np.asarray(v, np.float32).reshape(NT, 128).T)

    shared = {
        "wqT": np.ascontiguousarray(np.asarray(wq, np.float32).T),
        "wkT": np.ascontiguousarray(np.asarray(wk, np.float32).T),
        "wvT": np.ascontiguousarray(np.asarray(wv, np.float32).T),
        "wpT": np.ascontiguousarray(np.asarray(wp, np.float32).T),
        "gnw4": col4(gn_w),
        "gnb4": col4(gn_b),
        "bq4": col4(bq),
        "bk4": col4(bk),
        "bp4": col4(bp),
        "bv": np.ascontiguousarray(np.asarray(bv, np.float32).reshape(1, C)),
        "vones": np.ones((128, MT * NH), np.float32),
    }
    in_maps = [dict(shared, x=x_flat[i]) for i in range(B)]
    return in_maps


def kernel(x, gn_w, gn_b, wq, bq, wk, bk, wv, bv, wp, bp, **kw):
    nc = _get_nc()
    in_maps = _host_prep(x, gn_w, gn_b, wq, bq, wk, bk, wv, bv, wp, bp)
    res = run_bass_kernel_spmd(nc, in_maps, list(range(N_CORES)), **kw)
    out = np.stack([res.results[i]["out"] for i in range(N_CORES)])
    return out.reshape(8, C, 32, 32).astype(np.float32)


if __name__ == "__main__":
    rng = np.random.default_rng(0)
    xs_ = rng.standard_normal((8, C, 32, 32), dtype=np.float32)
    print("built nc ok:", _get_nc())
